# revision 16
# baseline (speedup 1.0000x reference)
"""GCC-PHAT Trainium2 kernel (v8: two-stage FFT forward, radix 128x8).

Pipeline (per core, batch-sharded B=16 -> 2 per core):
  1. Forward rfft as TWO PE stages (vs v7's direct [1024,1024] matmul):
     stage1: z[k2, n1] = DFT-128 over n2 of x[n1 + 8*n2] via S1 [128,128]
       (out rows: Re k2=0..64 at r=k2, Im k2=1..63 at r=64+k2).
     transpose: one DMA per (b, mic-pair) regroups z rows into 8 g-tiles
       with partitions (half, l, n1) -- conj bins fold into the same tile
       (row 64 conveniently holds Re z[64] where Im z[0] would sit).
     stage2: per chunk cp, 4 zero-half-padded stationaries S2 produce
       ps_a = Re X, ps_b = Im X for 128 f-bins (f-map fmap[cp][r]).
     PE cost drops ~2.2x vs direct DFT.
  2. PHAT normalize: ACT squares (fp16 out) + DVE add -> ACT rsqrt -> DVE
     muls ya/yb (PSUM src) and ys/yd (fp16).  DC bin handled on host.
  3. Pair products (28 pairs, diagonal pairing): Karatsuba 12 planes,
     split DVE/Pool by a static cost-balance table.
  4. Truncated inverse DFT, G stationary (rows permuted to fmap).
  5. PSUM -> ACT copy -> SBUF -> DMA out[b, lag, p, t].
"""

import os
from contextlib import ExitStack

import numpy as np

import concourse.bass as bass
import concourse.bacc as bacc
import concourse.mybir as mybir
import concourse.tile as tile
from concourse.bass import ds, ts
from concourse.bass_utils import run_bass_kernel_spmd

B, M, T, L = 16, 8, 250, 1024
NCORES = 8
NB = B // NCORES          # batches per core
NPAIRS = (M * (M - 1)) // 2   # 28
NL = 64                   # output lags
F32 = mybir.dt.float32
FP16 = mybir.dt.float16


def _f_of(g, l, u):
    if g == 0 and l == 0:
        return 128 * (u + 1) if u <= 3 else 64 + 128 * (u - 4)
    k2 = 8 * g + l
    return k2 + 128 * u if u <= 3 else 128 * (u - 3) - k2


def _fmap():
    fm = np.zeros((4, 128), dtype=int)
    for cp in range(4):
        for r in range(128):
            g = 2 * cp + (r >= 64)
            idx = r % 64
            fm[cp, r] = _f_of(g, idx // 8, idx % 8)
    return fm


def _build_S1() -> np.ndarray:
    """g-contiguous z layout: tile g = rows 16g..16g+15
    (Re k2=8g+l at 16g+l, Im k2 at 16g+8+l; row 8 holds Re z[64])."""
    S1 = np.zeros((128, 128))
    n2 = np.arange(128)
    for k2 in range(64):
        S1[:, 16 * (k2 // 8) + (k2 % 8)] = np.cos(2 * np.pi * k2 * n2 / 128)
    for k2 in range(1, 64):
        S1[:, 16 * (k2 // 8) + 8 + (k2 % 8)] = -np.sin(2 * np.pi * k2 * n2 / 128)
    S1[:, 8] = np.cos(2 * np.pi * 64 * n2 / 128)     # Re z[64]
    return S1.astype(np.float16)


def _build_S2() -> np.ndarray:
    """[16, 128, 128]: idx 4*cp + {0:RE_LO, 1:RE_HI, 2:IM_LO, 3:IM_HI}."""
    S2 = np.zeros((16, 128, 128))
    for cp in range(4):
        for hi, g in ((0, 2 * cp), (1, 2 * cp + 1)):
            half = 64 * hi
            re = S2[4 * cp + hi]
            im = S2[4 * cp + 2 + hi]
            for l in range(8):
                for u in range(8):
                    i = half + 8 * l + u
                    f = _f_of(g, l, u)
                    for n1 in range(8):
                        th = 2 * np.pi * f * n1 / L
                        c, s = np.cos(th), np.sin(th)
                        p_re = 8 * l + n1
                        p_im = 64 + 8 * l + n1
                        if g == 0 and l == 0:
                            if u <= 3:          # z0-derived
                                re[p_re, i] += c
                                im[p_re, i] += -s
                            else:               # z64-derived (lives on p_im)
                                re[p_im, i] += c
                                im[p_im, i] += -s
                        elif u <= 3:            # f mod 128 == k2
                            re[p_re, i] += c
                            re[p_im, i] += s
                            im[p_re, i] += -s
                            im[p_im, i] += c
                        else:                   # conj side
                            re[p_re, i] += c
                            re[p_im, i] += -s
                            im[p_re, i] += -s
                            im[p_im, i] += -c
    return S2.astype(np.float16)


def _build_G() -> np.ndarray:
    """12 inverse planes [128, 64]: idx cp = k1, 4+cp = k2, 8+cp = k3."""
    fm = _fmap()
    G = np.zeros((12, 128, NL), dtype=np.float64)
    nj = (np.arange(NL) - 32).astype(np.float64)
    for cp in range(4):
        for r in range(128):
            f = fm[cp, r]
            w = 1.0 if f == 512 else 2.0
            cosv = 16.0 * w * np.cos(2 * np.pi * f * nj / L) / L
            sinv = 16.0 * w * np.sin(2 * np.pi * f * nj / L) / L
            G[0 + cp, r] = cosv - sinv     # k1 = ys1*a2
            G[4 + cp, r] = sinv            # k2 = a1*ys2
            G[8 + cp, r] = -cosv           # k3 = b1*yd2
    return G.astype(np.float16)


def _pool_takes(kar, c, rows):
    """Static DVE/Pool balance for pair-product planes."""
    if kar == 2:
        return True                        # k3: all 4 chunks on Pool
    return False


def _prog_groups(mg):
    """Pair-product groups emitted after mic-group mg. Two-phase: the
    mics-0..3 groups go early (fills the DVE/Pool bubble during the
    forward), the rest in bulk 4-lane groups (min per-op overhead)."""
    if mg == 1:
        return [(1, 0, 3), (2, 0, 2), (3, 0, 1)]
    if mg == 3:
        return [(1, 3, 4), (2, 2, 4), (3, 1, 4), (4, 0, 4),
                (5, 0, 3), (6, 0, 2), (7, 0, 1)]
    return []


def build_bass() -> bass.Bass:
    nc = bacc.Bacc("TRN2", target_bir_lowering=False, debug=False)
    xP = nc.dram_tensor("xP", [NB, M, 128, 8, T], FP16, kind="ExternalInput")
    out = nc.dram_tensor("out", [NB, NL, NPAIRS, T], F32, kind="ExternalOutput")
    S1h = nc.inline_tensor(_build_S1(), name="S1mat")
    S2h = nc.inline_tensor(np.ascontiguousarray(_build_S2()), name="S2mat")
    Gh = nc.inline_tensor(np.ascontiguousarray(_build_G()), name="Gmat")

    with tile.TileContext(nc) as tc, ExitStack() as ctx:
        consts = ctx.enter_context(tc.tile_pool(name="consts", bufs=1))
        xt_pool = ctx.enter_context(tc.tile_pool(name="xt", bufs=2))
        zs_pool = ctx.enter_context(tc.tile_pool(name="zs", bufs=2))
        mv_pool = ctx.enter_context(tc.tile_pool(name="mv", bufs=2))
        y_pool = ctx.enter_context(tc.tile_pool(name="y", bufs=1))
        tmp_pool = ctx.enter_context(tc.tile_pool(name="tmp", bufs=2))
        r_pool = ctx.enter_context(tc.tile_pool(name="r", bufs=3))
        z_psum = ctx.enter_context(tc.tile_pool(name="zps", bufs=2, space="PSUM"))
        fwd_psum = ctx.enter_context(tc.tile_pool(name="fps", bufs=2, space="PSUM"))
        inv_psum = ctx.enter_context(tc.tile_pool(name="ips", bufs=2, space="PSUM"))

        s1_sb = consts.tile([128, 128], FP16)
        nc.sync.dma_start(s1_sb[:], S1h[:])
        s2_sb = consts.tile([128, 16, 128], FP16)
        nc.sync.dma_start(s2_sb[:], S2h[:].rearrange("i p j -> p i j"))
        g_sb = consts.tile([128, 12, NL], FP16)
        nc.sync.dma_start(g_sb[:], Gh[:].rearrange("i p j -> p i j"))

        for b in range(NB):
            # Y tiles: [128, mg(4), m(2), t] fp16 per (chunk, plane)
            ya = [y_pool.tile([128, 4, 2, T], FP16, tag=f"ya{c}", name=f"ya{c}") for c in range(4)]
            yb = [y_pool.tile([128, 4, 2, T], FP16, tag=f"yb{c}", name=f"yb{c}") for c in range(4)]
            ys = [y_pool.tile([128, 4, 2, T], FP16, tag=f"ys{c}", name=f"ys{c}") for c in range(4)]
            yd = [y_pool.tile([128, 4, 2, T], FP16, tag=f"yd{c}", name=f"yd{c}") for c in range(4)]

            # ---- forward (two-stage) + normalize ----
            for mg in range(4):
                xp_sb = xt_pool.tile([128, 2, 8, T], FP16, tag="xt")
                for mi in range(2):
                    nc.scalar.dma_start(xp_sb[:, mi], xP[b, 2 * mg + mi])
                # stage1: z rows on partitions, (n1, mi, t) free
                z_sb = zs_pool.tile([128, 8, 2, T], FP16, tag="z")
                for mi in range(2):
                    for q in range(4):      # n1-pair chunks
                        z_ps = z_psum.tile([128, 2, T], F32, tag="zp")
                        nc.tensor.matmul(
                            z_ps[:], s1_sb[:], xp_sb[:, mi, ts(q, 2)],
                            start=True, stop=True,
                        )
                        nc.scalar.copy(z_sb[:, ts(q, 2), mi], z_ps[:])
                # transpose: one DMA regroups all 8 g-tiles
                mov = mv_pool.tile([128, 8, 2, T], FP16, tag="mov")
                # transpose: tile g's 16 z-rows are contiguous partitions;
                # one DMA per g fans [16 rows x 8 n1] out to 128 partitions
                for g in range(8):
                    nc.sync.dma_start(mov[:, g], z_sb[:][16 * g:16 * g + 16])
                for cp in range(4):
                    ps_a = fwd_psum.tile([128, 2, T], F32, tag="psa")
                    ps_b = fwd_psum.tile([128, 2, T], F32, tag="psb")
                    nc.tensor.matmul(ps_a[:], s2_sb[:, 4 * cp + 0], mov[:, 2 * cp],
                                     start=True, stop=False)
                    nc.tensor.matmul(ps_a[:], s2_sb[:, 4 * cp + 1], mov[:, 2 * cp + 1],
                                     start=False, stop=True)
                    nc.tensor.matmul(ps_b[:], s2_sb[:, 4 * cp + 2], mov[:, 2 * cp],
                                     start=True, stop=False)
                    nc.tensor.matmul(ps_b[:], s2_sb[:, 4 * cp + 3], mov[:, 2 * cp + 1],
                                     start=False, stop=True)
                    # normalize (uniform across all rows); fp16 SBUF copies of
                    # ps_a/ps_b let every DVE op run in 2x mode
                    sq_a = tmp_pool.tile([128, 2, T], FP16, tag="sqa")
                    sq_b = tmp_pool.tile([128, 2, T], FP16, tag="sqb")
                    a16 = tmp_pool.tile([128, 2, T], FP16, tag="a16")
                    b16 = tmp_pool.tile([128, 2, T], FP16, tag="b16")
                    w = tmp_pool.tile([128, 2, T], FP16, tag="w")
                    nc.scalar.square(sq_a[:], ps_a[:])
                    nc.scalar.square(sq_b[:], ps_b[:])
                    nc.scalar.copy(a16[:], ps_a[:])
                    nc.scalar.copy(b16[:], ps_b[:])
                    nc.vector.tensor_add(sq_a[:], sq_a[:], sq_b[:])
                    # w' = 1/sqrt(16*r) = (1/|X|)/4
                    nc.scalar.activation(
                        w[:], sq_a[:],
                        mybir.ActivationFunctionType.Abs_reciprocal_sqrt,
                        scale=16.0,
                    )
                    nc.vector.tensor_mul(ya[cp][:, mg], a16[:], w[:])
                    nc.vector.tensor_mul(yb[cp][:, mg], b16[:], w[:])
                    nc.vector.tensor_add(ys[cp][:, mg], ya[cp][:, mg], yb[cp][:, mg])
                    nc.vector.tensor_sub(yd[cp][:, mg], ya[cp][:, mg], yb[cp][:, mg])

                # ---- pairs + inverse: emit groups as mics become ready ----
                yaf = [ya[c][:].rearrange("p a b t -> p (a b t)") for c in range(4)]
                ybf = [yb[c][:].rearrange("p a b t -> p (a b t)") for c in range(4)]
                ysf = [ys[c][:].rearrange("p a b t -> p (a b t)") for c in range(4)]
                ydf = [yd[c][:].rearrange("p a b t -> p (a b t)") for c in range(4)]
                for d, l0, lc in _prog_groups(mg):
                    kb = sum(M - dd for dd in range(1, d))
                    rows = lc * T
                    s1 = slice(l0 * T, l0 * T + rows)            # m1 side
                    s2 = slice((l0 + d) * T, (l0 + d) * T + rows)  # m2 side
                    r_sb = r_pool.tile([128, 12, 4 * T], FP16, tag="ru")
                    for c in range(4):
                        nc.vector.tensor_mul(r_sb[:, 0 + c, :rows], ysf[c][:, s1], yaf[c][:, s2])
                        if _pool_takes(1, c, rows):
                            nc.gpsimd.tensor_mul(r_sb[:, 4 + c, :rows], yaf[c][:, s1], ysf[c][:, s2])
                        else:
                            nc.vector.tensor_mul(r_sb[:, 4 + c, :rows], yaf[c][:, s1], ysf[c][:, s2])
                        if _pool_takes(2, c, rows):
                            nc.gpsimd.tensor_mul(r_sb[:, 8 + c, :rows], ybf[c][:, s1], ydf[c][:, s2])
                        else:
                            nc.vector.tensor_mul(r_sb[:, 8 + c, :rows], ybf[c][:, s1], ydf[c][:, s2])
                    for n0 in range(0, rows, 500):
                        nn = min(500, rows - n0)
                        ps_o = inv_psum.tile([64, 500], F32, tag="ops")
                        for idx in range(12):
                            nc.tensor.matmul(
                                ps_o[:, :nn],
                                g_sb[:, idx],
                                r_sb[:, idx, ds(n0, nn)],
                                start=(idx == 0), stop=(idx == 11),
                            )
                        o_sb = tmp_pool.tile([64, 2, T], F32, tag="osb")
                        nlanes = nn // T
                        src_ap = ps_o[:, :nn].rearrange("p (l t) -> p l t", t=T)
                        if (n0 // 500 + l0 + d) % 2:
                            nc.scalar.copy(o_sb[:, :nlanes], src_ap)
                        else:
                            nc.vector.tensor_copy(o_sb[:, :nlanes], src_ap)
                        nc.sync.dma_start(
                            out[b, :, ds(kb + l0 + n0 // T, nlanes)],
                            o_sb[:, :nlanes],
                        )
    nc.compile()
    return nc


_NC_CACHE = None


def kernel(x: np.ndarray) -> np.ndarray:
    global _NC_CACHE
    x = np.asarray(x, dtype=np.float32)
    assert x.shape == (B, M, T, L)
    # [B, M, L, T] -> split L = (n2:128, n1:8): xP[b,m,n2,n1,t] = x[b,m,t,8*n2+n1]
    xP = np.ascontiguousarray(
        x.transpose(0, 1, 3, 2).reshape(B, M, 128, 8, T)).astype(np.float16)
    s0 = np.sign(x.sum(axis=-1))  # [B, M, T] DC sign for host PHAT term
    if _NC_CACHE is None:
        _NC_CACHE = build_bass()
    nc = _NC_CACHE
    in_maps = [{"xP": xP[c * NB:(c + 1) * NB]} for c in range(NCORES)]
    trace = bool(int(os.environ.get("GCC_TRACE", "0")))
    res = run_bass_kernel_spmd(nc, in_maps, core_ids=list(range(NCORES)),
                               trace=trace)
    if trace and res.exec_time_ns is not None:
        print(f"HW exec time: {res.exec_time_ns} ns")
        if res.instructions_and_trace is not None:
            print("trace:", res.instructions_and_trace[1])
    out = np.concatenate([r["out"] for r in res.results], axis=0)  # [B,NL,28diag,T]
    plist = [m * (2 * M - m - 1) // 2 + (m + d - m - 1)
             for d in range(1, M) for m in range(M - d)]
    final = np.empty((B, NPAIRS, T, NL), dtype=np.float32)
    final[:, plist] = out.transpose(0, 2, 3, 1)
    # host DC (bin 0) PHAT term: sign(S1)*sign(S2)/L, constant over lags
    i1, i2 = np.triu_indices(M, k=1)
    final += (s0[:, i1] * s0[:, i2])[..., None].astype(np.float32) / L
    return final


# revision 21
# speedup vs baseline: 1.3729x; 1.3729x over previous
"""GCC-PHAT Trainium2 kernel (v8: two-stage FFT forward, radix 128x8).

Pipeline (per core, batch-sharded B=16 -> 2 per core):
  1. Forward rfft as TWO PE stages (vs v7's direct [1024,1024] matmul):
     stage1: z[k2, n1] = DFT-128 over n2 of x[n1 + 8*n2] via S1 [128,128]
       (out rows: Re k2=0..64 at r=k2, Im k2=1..63 at r=64+k2).
     transpose: one DMA per (b, mic-pair) regroups z rows into 8 g-tiles
       with partitions (half, l, n1) -- conj bins fold into the same tile
       (row 64 conveniently holds Re z[64] where Im z[0] would sit).
     stage2: per chunk cp, 4 zero-half-padded stationaries S2 produce
       ps_a = Re X, ps_b = Im X for 128 f-bins (f-map fmap[cp][r]).
     PE cost drops ~2.2x vs direct DFT.
  2. PHAT normalize: ACT squares (fp16 out) + DVE add -> ACT rsqrt -> DVE
     muls ya/yb (PSUM src) and ys/yd (fp16).  DC bin handled on host.
  3. Pair products (28 pairs, diagonal pairing): Karatsuba 12 planes,
     split DVE/Pool by a static cost-balance table.
  4. Truncated inverse DFT, G stationary (rows permuted to fmap).
  5. PSUM -> ACT copy -> SBUF -> DMA out[b, lag, p, t].
"""

import os
from contextlib import ExitStack

import numpy as np

import concourse.bass as bass
import concourse.bacc as bacc
import concourse.mybir as mybir
import concourse.tile as tile
from concourse.bass import ds, ts
from concourse.bass_utils import run_bass_kernel_spmd

B, M, T, L = 16, 8, 250, 1024
NCORES = 8
NB = B // NCORES          # batches per core
NPAIRS = (M * (M - 1)) // 2   # 28
NL = 64                   # output lags
F32 = mybir.dt.float32
FP16 = mybir.dt.float16


def _f_of(g, l, u):
    if g == 0 and l == 0:
        return 128 * (u + 1) if u <= 3 else 64 + 128 * (u - 4)
    k2 = 8 * g + l
    return k2 + 128 * u if u <= 3 else 128 * (u - 3) - k2


def _fmap():
    fm = np.zeros((4, 128), dtype=int)
    for cp in range(4):
        for r in range(128):
            g = 2 * cp + (r >= 64)
            idx = r % 64
            fm[cp, r] = _f_of(g, idx // 8, idx % 8)
    return fm


def _build_S1() -> np.ndarray:
    """g-contiguous z layout: tile g = rows 16g..16g+15
    (Re k2=8g+l at 16g+l, Im k2 at 16g+8+l; row 8 holds Re z[64])."""
    S1 = np.zeros((128, 128))
    n2 = np.arange(128)
    for k2 in range(64):
        S1[:, 16 * (k2 // 8) + (k2 % 8)] = np.cos(2 * np.pi * k2 * n2 / 128)
    for k2 in range(1, 64):
        S1[:, 16 * (k2 // 8) + 8 + (k2 % 8)] = -np.sin(2 * np.pi * k2 * n2 / 128)
    S1[:, 8] = np.cos(2 * np.pi * 64 * n2 / 128)     # Re z[64]
    return S1.astype(np.float16)


def _build_S2() -> np.ndarray:
    """[16, 128, 64]: idx 4*cp + {0:RE g0, 1:RE g1, 2:IM g0, 3:IM g1}.
    Each half-tile lands on ps partitions [0:64] or [64:128] via
    matmul tile_position (out.base_partition)."""
    S2 = np.zeros((16, 128, 64))
    for cp in range(4):
        for hi, g in ((0, 2 * cp), (1, 2 * cp + 1)):
            re = S2[4 * cp + hi]
            im = S2[4 * cp + 2 + hi]
            for l in range(8):
                for u in range(8):
                    i = 8 * l + u
                    f = _f_of(g, l, u)
                    for n1 in range(8):
                        th = 2 * np.pi * f * n1 / L
                        c, s = np.cos(th), np.sin(th)
                        p_re = 8 * l + n1
                        p_im = 64 + 8 * l + n1
                        if g == 0 and l == 0:
                            if u <= 3:          # z0-derived
                                re[p_re, i] += c
                                im[p_re, i] += -s
                            else:               # z64-derived (lives on p_im)
                                re[p_im, i] += c
                                im[p_im, i] += -s
                        elif u <= 3:            # f mod 128 == k2
                            re[p_re, i] += c
                            re[p_im, i] += s
                            im[p_re, i] += -s
                            im[p_im, i] += c
                        else:                   # conj side
                            re[p_re, i] += c
                            re[p_im, i] += -s
                            im[p_re, i] += -s
                            im[p_im, i] += -c
    return S2.astype(np.float16)


def _build_G() -> np.ndarray:
    """12 inverse planes [128, 64]: idx cp = k1, 4+cp = k2, 8+cp = k3."""
    fm = _fmap()
    G = np.zeros((12, 128, NL), dtype=np.float64)
    nj = (np.arange(NL) - 32).astype(np.float64)
    for cp in range(4):
        for r in range(128):
            f = fm[cp, r]
            w = 1.0 if f == 512 else 2.0
            cosv = 16.0 * w * np.cos(2 * np.pi * f * nj / L) / L
            sinv = 16.0 * w * np.sin(2 * np.pi * f * nj / L) / L
            G[0 + cp, r] = cosv - sinv     # k1 = ys1*a2
            G[4 + cp, r] = sinv            # k2 = a1*ys2
            G[8 + cp, r] = -cosv           # k3 = b1*yd2
    return G.astype(np.float16)


def _pool_takes(kar, c, rows):
    """Static DVE/Pool balance for pair-product planes."""
    if kar == 2:
        return True                        # k3: all 4 chunks on Pool
    return False


def _prog_groups(mg):
    """Pair-product groups emitted after mic-group mg. Two-phase: the
    mics-0..3 groups go early (fills the DVE/Pool bubble during the
    forward), the rest in bulk 4-lane groups (min per-op overhead)."""
    if mg == 1:
        return [(1, 0, 3), (2, 0, 2), (3, 0, 1)]
    if mg == 3:
        return [(1, 3, 4), (2, 2, 4), (3, 1, 4), (4, 0, 4),
                (5, 0, 3), (6, 0, 2), (7, 0, 1)]
    return []


def build_bass() -> bass.Bass:
    nc = bacc.Bacc("TRN2", target_bir_lowering=False, debug=False)
    xP = nc.dram_tensor("xP", [NB, M, 128, 8, T], FP16, kind="ExternalInput")
    out = nc.dram_tensor("out", [NB, NL, NPAIRS, T], F32, kind="ExternalOutput")
    S1h = nc.inline_tensor(_build_S1(), name="S1mat")
    S2h = nc.inline_tensor(np.ascontiguousarray(_build_S2()), name="S2mat")
    Gh = nc.inline_tensor(np.ascontiguousarray(_build_G()), name="Gmat")

    with tile.TileContext(nc) as tc, ExitStack() as ctx:
        consts = ctx.enter_context(tc.tile_pool(name="consts", bufs=1))
        xt_pool = ctx.enter_context(tc.tile_pool(name="xt", bufs=2))
        zs_pool = ctx.enter_context(tc.tile_pool(name="zs", bufs=2))
        mv_pool = ctx.enter_context(tc.tile_pool(name="mv", bufs=2))
        y_pool = ctx.enter_context(tc.tile_pool(name="y", bufs=1))
        tmp_pool = ctx.enter_context(tc.tile_pool(name="tmp", bufs=2))
        r_pool = ctx.enter_context(tc.tile_pool(name="r", bufs=3))
        z_psum = ctx.enter_context(tc.tile_pool(name="zps", bufs=2, space="PSUM"))
        fwd_psum = ctx.enter_context(tc.tile_pool(name="fps", bufs=2, space="PSUM"))
        inv_psum = ctx.enter_context(tc.tile_pool(name="ips", bufs=2, space="PSUM"))

        s1_sb = consts.tile([128, 128], FP16)
        nc.sync.dma_start(s1_sb[:], S1h[:])
        s2_sb = consts.tile([128, 16, 64], FP16)
        nc.sync.dma_start(s2_sb[:], S2h[:].rearrange("i p j -> p i j"))
        g_sb = consts.tile([128, 12, NL], FP16)
        nc.sync.dma_start(g_sb[:], Gh[:].rearrange("i p j -> p i j"))

        for b in range(NB):
            # Y tiles: [128, mg(4), m(2), t] fp16 per (chunk, plane)
            ya = [y_pool.tile([128, 4, 2, T], FP16, tag=f"ya{c}", name=f"ya{c}") for c in range(4)]
            yb = [y_pool.tile([128, 4, 2, T], FP16, tag=f"yb{c}", name=f"yb{c}") for c in range(4)]
            ys = [y_pool.tile([128, 4, 2, T], FP16, tag=f"ys{c}", name=f"ys{c}") for c in range(4)]
            yd = [y_pool.tile([128, 4, 2, T], FP16, tag=f"yd{c}", name=f"yd{c}") for c in range(4)]

            # ---- forward (two-stage) + normalize ----
            for mg in range(4):
                xp_sb = xt_pool.tile([128, 2, 8, T], FP16, tag="xt")
                for mi in range(2):
                    nc.scalar.dma_start(xp_sb[:, mi], xP[b, 2 * mg + mi])
                # stage1: z rows on partitions, (n1, mi, t) free
                z_sb = zs_pool.tile([128, 8, 2, T], FP16, tag="z")
                for mi in range(2):
                    for q in range(4):      # n1-pair chunks
                        z_ps = z_psum.tile([128, 2, T], F32, tag="zp")
                        nc.tensor.matmul(
                            z_ps[:], s1_sb[:], xp_sb[:, mi, ts(q, 2)],
                            start=True, stop=True,
                        )
                        nc.scalar.copy(z_sb[:, ts(q, 2), mi], z_ps[:])
                # transpose: one DMA regroups all 8 g-tiles
                mov = mv_pool.tile([128, 8, 2, T], FP16, tag="mov")
                # transpose: tile g's 16 z-rows are contiguous partitions;
                # one DMA per g fans [16 rows x 8 n1] out to 128 partitions
                for g in range(8):
                    nc.sync.dma_start(mov[:, g], z_sb[:][16 * g:16 * g + 16])
                for cp in range(4):
                    ps_a = fwd_psum.tile([128, 2, T], F32, tag="psa")
                    ps_b = fwd_psum.tile([128, 2, T], F32, tag="psb")
                    nc.tensor.matmul(ps_a[:][0:64], s2_sb[:, 4 * cp + 0],
                                     mov[:, 2 * cp], start=True, stop=True)
                    nc.tensor.matmul(ps_a[:][64:128], s2_sb[:, 4 * cp + 1],
                                     mov[:, 2 * cp + 1], start=True, stop=True)
                    nc.tensor.matmul(ps_b[:][0:64], s2_sb[:, 4 * cp + 2],
                                     mov[:, 2 * cp], start=True, stop=True)
                    nc.tensor.matmul(ps_b[:][64:128], s2_sb[:, 4 * cp + 3],
                                     mov[:, 2 * cp + 1], start=True, stop=True)
                    # normalize (uniform across all rows); PSUM-sourced muls
                    # keep SBUF traffic low (PSUM reads use a separate port)
                    sq_a = tmp_pool.tile([128, 2, T], FP16, tag="sqa")
                    sq_b = tmp_pool.tile([128, 2, T], FP16, tag="sqb")
                    w = tmp_pool.tile([128, 2, T], F32, tag="w")
                    nc.scalar.square(sq_a[:], ps_a[:])
                    nc.scalar.square(sq_b[:], ps_b[:])
                    nc.vector.tensor_add(sq_a[:], sq_a[:], sq_b[:])
                    # w' = 1/sqrt(16*r) = (1/|X|)/4
                    nc.scalar.activation(
                        w[:], sq_a[:],
                        mybir.ActivationFunctionType.Abs_reciprocal_sqrt,
                        scale=16.0,
                    )
                    nc.vector.tensor_mul(ya[cp][:, mg], ps_a[:], w[:])
                    nc.vector.tensor_mul(yb[cp][:, mg], ps_b[:], w[:])
                    nc.vector.tensor_add(ys[cp][:, mg], ya[cp][:, mg], yb[cp][:, mg])
                    nc.vector.tensor_sub(yd[cp][:, mg], ya[cp][:, mg], yb[cp][:, mg])

                # ---- pairs + inverse: emit groups as mics become ready ----
                yaf = [ya[c][:].rearrange("p a b t -> p (a b t)") for c in range(4)]
                ybf = [yb[c][:].rearrange("p a b t -> p (a b t)") for c in range(4)]
                ysf = [ys[c][:].rearrange("p a b t -> p (a b t)") for c in range(4)]
                ydf = [yd[c][:].rearrange("p a b t -> p (a b t)") for c in range(4)]
                for d, l0, lc in _prog_groups(mg):
                    kb = sum(M - dd for dd in range(1, d))
                    rows = lc * T
                    s1 = slice(l0 * T, l0 * T + rows)            # m1 side
                    s2 = slice((l0 + d) * T, (l0 + d) * T + rows)  # m2 side
                    r_sb = r_pool.tile([128, 12, 4 * T], FP16, tag="ru")
                    for c in range(4):
                        nc.vector.tensor_mul(r_sb[:, 0 + c, :rows], ysf[c][:, s1], yaf[c][:, s2])
                        if _pool_takes(1, c, rows):
                            nc.gpsimd.tensor_mul(r_sb[:, 4 + c, :rows], yaf[c][:, s1], ysf[c][:, s2])
                        else:
                            nc.vector.tensor_mul(r_sb[:, 4 + c, :rows], yaf[c][:, s1], ysf[c][:, s2])
                        if _pool_takes(2, c, rows):
                            nc.gpsimd.tensor_mul(r_sb[:, 8 + c, :rows], ybf[c][:, s1], ydf[c][:, s2])
                        else:
                            nc.vector.tensor_mul(r_sb[:, 8 + c, :rows], ybf[c][:, s1], ydf[c][:, s2])
                    for n0 in range(0, rows, 500):
                        nn = min(500, rows - n0)
                        ps_o = inv_psum.tile([64, 500], F32, tag="ops")
                        for idx in range(12):
                            nc.tensor.matmul(
                                ps_o[:, :nn],
                                g_sb[:, idx],
                                r_sb[:, idx, ds(n0, nn)],
                                start=(idx == 0), stop=(idx == 11),
                            )
                        o_sb = tmp_pool.tile([64, 2, T], F32, tag="osb")
                        nlanes = nn // T
                        nc.scalar.copy(
                            o_sb[:, :nlanes],
                            ps_o[:, :nn].rearrange("p (l t) -> p l t", t=T),
                        )
                        nc.sync.dma_start(
                            out[b, :, ds(kb + l0 + n0 // T, nlanes)],
                            o_sb[:, :nlanes],
                        )
    nc.compile()
    return nc


_NC_CACHE = None


def kernel(x: np.ndarray) -> np.ndarray:
    global _NC_CACHE
    x = np.asarray(x, dtype=np.float32)
    assert x.shape == (B, M, T, L)
    # [B, M, L, T] -> split L = (n2:128, n1:8): xP[b,m,n2,n1,t] = x[b,m,t,8*n2+n1]
    xP = np.ascontiguousarray(
        x.transpose(0, 1, 3, 2).reshape(B, M, 128, 8, T)).astype(np.float16)
    s0 = np.sign(x.sum(axis=-1))  # [B, M, T] DC sign for host PHAT term
    if _NC_CACHE is None:
        _NC_CACHE = build_bass()
    nc = _NC_CACHE
    in_maps = [{"xP": xP[c * NB:(c + 1) * NB]} for c in range(NCORES)]
    trace = bool(int(os.environ.get("GCC_TRACE", "0")))
    res = run_bass_kernel_spmd(nc, in_maps, core_ids=list(range(NCORES)),
                               trace=trace)
    if trace and res.exec_time_ns is not None:
        print(f"HW exec time: {res.exec_time_ns} ns")
        if res.instructions_and_trace is not None:
            print("trace:", res.instructions_and_trace[1])
    out = np.concatenate([r["out"] for r in res.results], axis=0)  # [B,NL,28diag,T]
    plist = [m * (2 * M - m - 1) // 2 + (m + d - m - 1)
             for d in range(1, M) for m in range(M - d)]
    final = np.empty((B, NPAIRS, T, NL), dtype=np.float32)
    final[:, plist] = out.transpose(0, 2, 3, 1)
    # host DC (bin 0) PHAT term: sign(S1)*sign(S2)/L, constant over lags
    i1, i2 = np.triu_indices(M, k=1)
    final += (s0[:, i1] * s0[:, i2])[..., None].astype(np.float32) / L
    return final
